# revision 35
# baseline (speedup 1.0000x reference)
"""Trainium2 Bass kernel: batched masked mean pooling (segment_reduce).

Computes out[b,e,d] = sum_l(entity_mapping[b,e,l] * doc_state[b,l,d]) / entity_lens[b,e]
for B=8, E=512, L=4096, D=256 — i.e. per batch b a 512x4096 @ 4096x256 GEMM
plus a per-row divide.

Sharding: data-parallel over batch B — one batch per NeuronCore (B=8 = 8 cores).

Per-core kernel:
  - doc_state[b] is loaded once into SBUF with L on partitions ([128, 32, 256]).
  - entity_mapping[b] is loaded in natural [E,L] row-blocks (contiguous DMA),
    transposed on the TensorEngine in 128x128 tiles (exact permutation),
    and used as the stationary matmul operand with L on partitions.
  - 32 accumulating matmuls per 128-row E-tile produce PSUM [128, 256],
    which is scaled by 1/entity_lens on the VectorEngine and DMA'd out.
  - Matmuls run as float32r (single-pass fp32 streaming, 4x the fp32 matmul
    rate); set MEANPOOL_DTYPE=float32 for the exact two-pass path.
"""

import os
import sys

for _p in ("/opt/trn_rl_repo", "/root/.axon_site/_ro/trn_rl_repo"):
    if os.path.isdir(_p) and _p not in sys.path:
        sys.path.insert(0, _p)

import numpy as np
from contextlib import ExitStack

import concourse.bass as bass
import concourse.tile as tile
from concourse import bacc, mybir
from concourse import bass_utils
from concourse.masks import make_identity

B, E, L, D = 8, 512, 4096, 256
P = 128
NK = L // P  # 32 k-tiles along the contraction dim
NE = E // P  # 4 e-tiles (output partition tiles)
N_CORES = 8

_DTYPE_NAME = os.environ.get("MEANPOOL_DTYPE", "float32r")
MM_DT = getattr(mybir.dt, _DTYPE_NAME)
F32 = mybir.dt.float32

# DMA chunking: k-tiles per map/doc DMA (4KB contiguous per partition per chunk)
KCH = 8


def _build_bass():
    nc = bacc.Bacc("TRN2", target_bir_lowering=False, debug=False)

    map_d = nc.dram_tensor("map", [E, L], MM_DT, kind="ExternalInput").ap()
    doc_d = nc.dram_tensor("doc", [L, D], MM_DT, kind="ExternalInput").ap()
    lens_d = nc.dram_tensor("lens", [P, NE], F32, kind="ExternalInput").ap()
    out_d = nc.dram_tensor("out", [E, D], F32, kind="ExternalOutput").ap()

    NCH = NK // KCH  # 4 chunk-groups over the contraction dim

    with tile.TileContext(nc) as tc:
        with ExitStack() as ctx:
            consts = ctx.enter_context(tc.tile_pool(name="consts", bufs=1))
            docp = ctx.enter_context(tc.tile_pool(name="docp", bufs=1))
            mapnat = ctx.enter_context(tc.tile_pool(name="mapnat", bufs=2 * NE))
            mapt = ctx.enter_context(tc.tile_pool(name="mapt", bufs=6))
            outp = ctx.enter_context(tc.tile_pool(name="outp", bufs=2))
            pst = ctx.enter_context(tc.tile_pool(name="pst", bufs=4, space="PSUM"))
            pso = ctx.enter_context(tc.tile_pool(name="pso", bufs=1, space="PSUM"))

            # identity for PE transposes; gpsimd can't write f32r, so build it
            # in f32 and round via a DVE copy (0/1 round exactly).
            ident_f = consts.tile([P, P], F32)
            make_identity(nc, ident_f[:])
            if MM_DT == F32:
                ident = ident_f
            else:
                ident = consts.tile([P, P], MM_DT)
                nc.vector.tensor_copy(ident[:], ident_f[:])

            lens_sb = consts.tile([P, NE], F32)
            nc.gpsimd.dma_start(lens_sb[:], lens_d[:])
            recip = consts.tile([P, NE], F32)
            nc.vector.reciprocal(recip[:], lens_sb[:])

            # doc_state with L on partitions: [p, k, d]
            doc_sb = docp.tile([P, NK, D], MM_DT)
            doc_r = doc_d.rearrange("(k p) d -> p k d", p=P)

            map_rs = [
                map_d[et * P : (et + 1) * P, :].rearrange("e (k l) -> e k l", l=P)
                for et in range(NE)
            ]

            # One PSUM accumulation group per e-tile, all four open across the
            # whole chunk sweep (4 banks for outputs + 4 for transposes).
            ps_os = [
                pso.tile([P, D], F32, name=f"pso{et}", tag=f"pso{et}")
                for et in range(NE)
            ]

            # Pipeline over chunk-groups: within group c, DMA the four 512KB
            # map blocks (et, c) and the doc chunk, then transpose + matmul
            # each e-tile's chunk. ALL input loads go on the sync engine:
            # HWDGE DMAs execute FIFO per issuing engine, so a single queue
            # delivers data in exact consumption order at full aggregate
            # bandwidth (one InstDMACopy spans all 16 SDMA engines) — no
            # fair-share mixing. Dependent stores (out) go on gpsimd so they
            # never head-of-line-block the load stream.
            # tapered chunk-groups: small first group so the PE pipeline
            # fills with ~1.5MB in flight instead of 3MB, small last group so
            # the final matmul burst (and the stores behind it) ends sooner.
            GROUPS = [4, 8, 8, 8, 4]
            assert sum(GROUPS) == NK
            k0 = 0
            for c, g in enumerate(GROUPS):
                ksl = slice(k0, k0 + g)
                mn_c = []
                h = g // 2
                for et in range(NE):
                    mn = mapnat.tile(
                        [P, g, P], MM_DT, name=f"mn{et}_{c}", tag="mn"
                    )
                    if c == 0 and et == 0:
                        # split the very first block so PE can start sooner
                        nc.sync.dma_start(mn[:, :2, :], map_rs[0][:, k0 : k0 + 2, :])
                        nc.sync.dma_start(mn[:, 2:g, :], map_rs[0][:, k0 + 2 : k0 + g, :])
                    else:
                        nc.sync.dma_start(mn[:], map_rs[et][:, ksl, :])
                    mn_c.append(mn)
                    # interleave doc halves so the first matmuls of the group
                    # don't wait behind a full doc transfer in the FIFO
                    if et == 0:
                        dsl = slice(k0, k0 + h)
                        nc.sync.dma_start(doc_sb[:, dsl, :], doc_r[:, dsl, :])
                    elif et == 1:
                        dsl = slice(k0 + h, k0 + g)
                        nc.sync.dma_start(doc_sb[:, dsl, :], doc_r[:, dsl, :])

                for et in range(NE):
                    mn = mn_c[et]
                    mt = mapt.tile([P, g, P], MM_DT, name=f"mt{et}_{c}", tag="mt")
                    for j in range(g):
                        ps_t = pst.tile([P, P], MM_DT)
                        nc.tensor.transpose(ps_t[:], mn[:, j, :], ident[:])
                        nc.vector.tensor_copy(mt[:, j, :], ps_t[:])
                    for j in range(g):
                        nc.tensor.matmul(
                            ps_os[et][:],
                            mt[:, j, :],
                            doc_sb[:, k0 + j, :],
                            start=(c == 0 and j == 0),
                            stop=(c == len(GROUPS) - 1 and j == g - 1),
                        )
                k0 += g

            # scale by 1/lens (per-partition scalar) and store
            for et in range(NE):
                ob = outp.tile([P, D], F32, name=f"ob{et}", tag="ob")
                nc.vector.tensor_scalar_mul(ob[:], ps_os[et][:], recip[:, et : et + 1])
                eng = nc.scalar if et % 2 == 0 else nc.sync
                eng.dma_start(out_d[et * P : (et + 1) * P, :], ob[:])

    nc.compile()
    return nc


_NC_CACHE = {}


def _get_nc():
    if "nc" not in _NC_CACHE:
        _NC_CACHE["nc"] = _build_bass()
    return _NC_CACHE["nc"]


def kernel(doc_state, entity_mapping, entity_lens, _trace=False, _tmpdir=None):
    doc_state = np.asarray(doc_state, dtype=np.float32)
    entity_mapping = np.asarray(entity_mapping, dtype=np.float32)
    entity_lens = np.asarray(entity_lens, dtype=np.float32)
    assert doc_state.shape == (B, L, D)
    assert entity_mapping.shape == (B, E, L)
    assert entity_lens.shape == (B, E)

    nc = _get_nc()

    in_maps = []
    for b in range(B):
        in_maps.append(
            {
                "map": np.ascontiguousarray(entity_mapping[b]),
                "doc": np.ascontiguousarray(doc_state[b]),
                # packed so lens[e] sits at partition e%128, column e//128
                "lens": np.ascontiguousarray(entity_lens[b].reshape(NE, P).T),
            }
        )

    res = bass_utils.run_bass_kernel_spmd(
        nc,
        in_maps,
        core_ids=list(range(N_CORES)),
        trace=_trace,
        tmpdir=_tmpdir,
    )
    out = np.stack([res.results[b]["out"] for b in range(B)], axis=0)
    if _trace:
        kernel.last_result = res
    return out


# revision 36
# speedup vs baseline: 1.0440x; 1.0440x over previous
"""Trainium2 Bass kernel: batched masked mean pooling (segment_reduce).

Computes out[b,e,d] = sum_l(entity_mapping[b,e,l] * doc_state[b,l,d]) / entity_lens[b,e]
for B=8, E=512, L=4096, D=256 — i.e. per batch b a 512x4096 @ 4096x256 GEMM
plus a per-row divide.

Sharding: data-parallel over batch B — one batch per NeuronCore (B=8 = 8 cores).

Per-core kernel:
  - doc_state[b] is loaded once into SBUF with L on partitions ([128, 32, 256]).
  - entity_mapping[b] is loaded in natural [E,L] row-blocks (contiguous DMA),
    transposed on the TensorEngine in 128x128 tiles (exact permutation),
    and used as the stationary matmul operand with L on partitions.
  - 32 accumulating matmuls per 128-row E-tile produce PSUM [128, 256],
    which is scaled by 1/entity_lens on the VectorEngine and DMA'd out.
  - Matmuls run as float32r (single-pass fp32 streaming, 4x the fp32 matmul
    rate); set MEANPOOL_DTYPE=float32 for the exact two-pass path.
"""

import os
import sys

for _p in ("/opt/trn_rl_repo", "/root/.axon_site/_ro/trn_rl_repo"):
    if os.path.isdir(_p) and _p not in sys.path:
        sys.path.insert(0, _p)

import numpy as np
from contextlib import ExitStack

import concourse.bass as bass
import concourse.tile as tile
from concourse import bacc, mybir
from concourse import bass_utils
from concourse.masks import make_identity

B, E, L, D = 8, 512, 4096, 256
P = 128
NK = L // P  # 32 k-tiles along the contraction dim
NE = E // P  # 4 e-tiles (output partition tiles)
N_CORES = 8

_DTYPE_NAME = os.environ.get("MEANPOOL_DTYPE", "float32r")
MM_DT = getattr(mybir.dt, _DTYPE_NAME)
F32 = mybir.dt.float32

# DMA chunking: k-tiles per map/doc DMA (4KB contiguous per partition per chunk)
KCH = 8


def _build_bass():
    nc = bacc.Bacc("TRN2", target_bir_lowering=False, debug=False)

    map_d = nc.dram_tensor("map", [E, L], MM_DT, kind="ExternalInput").ap()
    doc_d = nc.dram_tensor("doc", [L, D], MM_DT, kind="ExternalInput").ap()
    lens_d = nc.dram_tensor("lens", [P, NE], F32, kind="ExternalInput").ap()
    out_d = nc.dram_tensor("out", [E, D], F32, kind="ExternalOutput").ap()

    NCH = NK // KCH  # 4 chunk-groups over the contraction dim

    with tile.TileContext(nc) as tc:
        with ExitStack() as ctx:
            consts = ctx.enter_context(tc.tile_pool(name="consts", bufs=1))
            docp = ctx.enter_context(tc.tile_pool(name="docp", bufs=1))
            mapnat = ctx.enter_context(tc.tile_pool(name="mapnat", bufs=2 * NE))
            mapt = ctx.enter_context(tc.tile_pool(name="mapt", bufs=6))
            outp = ctx.enter_context(tc.tile_pool(name="outp", bufs=2))
            pst = ctx.enter_context(tc.tile_pool(name="pst", bufs=4, space="PSUM"))
            pso = ctx.enter_context(tc.tile_pool(name="pso", bufs=1, space="PSUM"))

            # identity for PE transposes; gpsimd can't write f32r, so build it
            # in f32 and round via a DVE copy (0/1 round exactly).
            ident_f = consts.tile([P, P], F32)
            make_identity(nc, ident_f[:])
            if MM_DT == F32:
                ident = ident_f
            else:
                ident = consts.tile([P, P], MM_DT)
                nc.vector.tensor_copy(ident[:], ident_f[:])

            lens_sb = consts.tile([P, NE], F32)
            nc.gpsimd.dma_start(lens_sb[:], lens_d[:])
            recip = consts.tile([P, NE], F32)
            nc.vector.reciprocal(recip[:], lens_sb[:])

            # doc_state with L on partitions: [p, k, d]
            doc_sb = docp.tile([P, NK, D], MM_DT)
            doc_r = doc_d.rearrange("(k p) d -> p k d", p=P)

            map_rs = [
                map_d[et * P : (et + 1) * P, :].rearrange("e (k l) -> e k l", l=P)
                for et in range(NE)
            ]

            # One PSUM accumulation group per e-tile, all four open across the
            # whole chunk sweep (4 banks for outputs + 4 for transposes).
            ps_os = [
                pso.tile([P, D], F32, name=f"pso{et}", tag=f"pso{et}")
                for et in range(NE)
            ]

            # Pipeline over chunk-groups: within group c, DMA the four 512KB
            # map blocks (et, c) and the doc chunk, then transpose + matmul
            # each e-tile's chunk. ALL input loads go on the sync engine:
            # HWDGE DMAs execute FIFO per issuing engine, so a single queue
            # delivers data in exact consumption order at full aggregate
            # bandwidth (one InstDMACopy spans all 16 SDMA engines) — no
            # fair-share mixing. Dependent stores (out) go on gpsimd so they
            # never head-of-line-block the load stream.
            for c in range(NCH):
                ksl = slice(c * KCH, (c + 1) * KCH)
                mn_c = []
                h = KCH // 2
                for et in range(NE):
                    mn = mapnat.tile(
                        [P, KCH, P], MM_DT, name=f"mn{et}_{c}", tag="mn"
                    )
                    if c == 0 and et == 0:
                        # split the very first block so PE can start sooner
                        nc.sync.dma_start(mn[:, :2, :], map_rs[0][:, :2, :])
                        nc.sync.dma_start(mn[:, 2:KCH, :], map_rs[0][:, 2:KCH, :])
                    else:
                        nc.sync.dma_start(mn[:], map_rs[et][:, ksl, :])
                    mn_c.append(mn)
                    # interleave doc halves so the first matmuls of the group
                    # don't wait behind a full 1MB doc transfer in the FIFO
                    if et == 0:
                        dsl = slice(c * KCH, c * KCH + h)
                        nc.sync.dma_start(doc_sb[:, dsl, :], doc_r[:, dsl, :])
                    elif et == 1:
                        dsl = slice(c * KCH + h, (c + 1) * KCH)
                        nc.sync.dma_start(doc_sb[:, dsl, :], doc_r[:, dsl, :])

                for et in range(NE):
                    mn = mn_c[et]
                    mt = mapt.tile([P, KCH, P], MM_DT, name=f"mt{et}_{c}", tag="mt")
                    for j in range(KCH):
                        ps_t = pst.tile([P, P], MM_DT)
                        nc.tensor.transpose(ps_t[:], mn[:, j, :], ident[:])
                        nc.vector.tensor_copy(mt[:, j, :], ps_t[:])
                    for j in range(KCH):
                        nc.tensor.matmul(
                            ps_os[et][:],
                            mt[:, j, :],
                            doc_sb[:, c * KCH + j, :],
                            start=(c == 0 and j == 0),
                            stop=(c == NCH - 1 and j == KCH - 1),
                        )

            # scale by 1/lens (per-partition scalar) and store
            for et in range(NE):
                ob = outp.tile([P, D], F32, name=f"ob{et}", tag="ob")
                nc.vector.tensor_scalar_mul(ob[:], ps_os[et][:], recip[:, et : et + 1])
                eng = nc.scalar if et % 2 == 0 else nc.sync
                eng.dma_start(out_d[et * P : (et + 1) * P, :], ob[:])

    nc.compile()
    return nc


_NC_CACHE = {}


def _get_nc():
    if "nc" not in _NC_CACHE:
        _NC_CACHE["nc"] = _build_bass()
    return _NC_CACHE["nc"]


def kernel(doc_state, entity_mapping, entity_lens, _trace=False, _tmpdir=None):
    doc_state = np.asarray(doc_state, dtype=np.float32)
    entity_mapping = np.asarray(entity_mapping, dtype=np.float32)
    entity_lens = np.asarray(entity_lens, dtype=np.float32)
    assert doc_state.shape == (B, L, D)
    assert entity_mapping.shape == (B, E, L)
    assert entity_lens.shape == (B, E)

    nc = _get_nc()

    in_maps = []
    for b in range(B):
        in_maps.append(
            {
                "map": np.ascontiguousarray(entity_mapping[b]),
                "doc": np.ascontiguousarray(doc_state[b]),
                # packed so lens[e] sits at partition e%128, column e//128
                "lens": np.ascontiguousarray(entity_lens[b].reshape(NE, P).T),
            }
        )

    res = bass_utils.run_bass_kernel_spmd(
        nc,
        in_maps,
        core_ids=list(range(N_CORES)),
        trace=_trace,
        tmpdir=_tmpdir,
    )
    out = np.stack([res.results[b]["out"] for b in range(B)], axis=0)
    if _trace:
        kernel.last_result = res
    return out


# revision 37
# speedup vs baseline: 1.0754x; 1.0301x over previous
"""Trainium2 Bass kernel: batched masked mean pooling (segment_reduce).

Computes out[b,e,d] = sum_l(entity_mapping[b,e,l] * doc_state[b,l,d]) / entity_lens[b,e]
for B=8, E=512, L=4096, D=256 — i.e. per batch b a 512x4096 @ 4096x256 GEMM
plus a per-row divide.

Sharding: data-parallel over batch B — one batch per NeuronCore (B=8 = 8 cores).

Per-core kernel:
  - doc_state[b] is loaded once into SBUF with L on partitions ([128, 32, 256]).
  - entity_mapping[b] is loaded in natural [E,L] row-blocks (contiguous DMA),
    transposed on the TensorEngine in 128x128 tiles (exact permutation),
    and used as the stationary matmul operand with L on partitions.
  - 32 accumulating matmuls per 128-row E-tile produce PSUM [128, 256],
    which is scaled by 1/entity_lens on the VectorEngine and DMA'd out.
  - Matmuls run as float32r (single-pass fp32 streaming, 4x the fp32 matmul
    rate); set MEANPOOL_DTYPE=float32 for the exact two-pass path.
"""

import os
import sys

for _p in ("/opt/trn_rl_repo", "/root/.axon_site/_ro/trn_rl_repo"):
    if os.path.isdir(_p) and _p not in sys.path:
        sys.path.insert(0, _p)

import numpy as np
from contextlib import ExitStack

import concourse.bass as bass
import concourse.tile as tile
from concourse import bacc, mybir
from concourse import bass_utils
from concourse.masks import make_identity

B, E, L, D = 8, 512, 4096, 256
P = 128
NK = L // P  # 32 k-tiles along the contraction dim
NE = E // P  # 4 e-tiles (output partition tiles)
N_CORES = 8

_DTYPE_NAME = os.environ.get("MEANPOOL_DTYPE", "float32r")
MM_DT = getattr(mybir.dt, _DTYPE_NAME)
F32 = mybir.dt.float32

# DMA chunking: k-tiles per map/doc DMA (4KB contiguous per partition per chunk)
KCH = 8


def _build_bass():
    nc = bacc.Bacc("TRN2", target_bir_lowering=False, debug=False)

    map_d = nc.dram_tensor("map", [E, L], MM_DT, kind="ExternalInput").ap()
    doc_d = nc.dram_tensor("doc", [L, D], MM_DT, kind="ExternalInput").ap()
    lens_d = nc.dram_tensor("lens", [P, NE], F32, kind="ExternalInput").ap()
    out_d = nc.dram_tensor("out", [E, D], F32, kind="ExternalOutput").ap()

    NCH = NK // KCH  # 4 chunk-groups over the contraction dim

    with tile.TileContext(nc) as tc:
        with ExitStack() as ctx:
            consts = ctx.enter_context(tc.tile_pool(name="consts", bufs=1))
            docp = ctx.enter_context(tc.tile_pool(name="docp", bufs=1))
            mapnat = ctx.enter_context(tc.tile_pool(name="mapnat", bufs=2 * NE))
            mapt = ctx.enter_context(tc.tile_pool(name="mapt", bufs=6))
            outp = ctx.enter_context(tc.tile_pool(name="outp", bufs=4))
            pst = ctx.enter_context(tc.tile_pool(name="pst", bufs=4, space="PSUM"))
            pso = ctx.enter_context(tc.tile_pool(name="pso", bufs=1, space="PSUM"))

            # identity for PE transposes; gpsimd can't write f32r, so build it
            # in f32 and round via a DVE copy (0/1 round exactly).
            ident_f = consts.tile([P, P], F32)
            make_identity(nc, ident_f[:])
            if MM_DT == F32:
                ident = ident_f
            else:
                ident = consts.tile([P, P], MM_DT)
                nc.vector.tensor_copy(ident[:], ident_f[:])

            lens_sb = consts.tile([P, NE], F32)
            nc.gpsimd.dma_start(lens_sb[:], lens_d[:])
            recip = consts.tile([P, NE], F32)
            nc.vector.reciprocal(recip[:], lens_sb[:])

            # doc_state with L on partitions: [p, k, d]
            doc_sb = docp.tile([P, NK, D], MM_DT)
            doc_r = doc_d.rearrange("(k p) d -> p k d", p=P)

            map_rs = [
                map_d[et * P : (et + 1) * P, :].rearrange("e (k l) -> e k l", l=P)
                for et in range(NE)
            ]

            # One PSUM accumulation group per e-tile, all four open across the
            # whole chunk sweep (4 banks for outputs + 4 for transposes).
            ps_os = [
                pso.tile([P, D], F32, name=f"pso{et}", tag=f"pso{et}")
                for et in range(NE)
            ]

            # Pipeline over chunk-groups: within group c, DMA the four 512KB
            # map blocks (et, c) and the doc chunk, then transpose + matmul
            # each e-tile's chunk. ALL input loads go on the sync engine:
            # HWDGE DMAs execute FIFO per issuing engine, so a single queue
            # delivers data in exact consumption order at full aggregate
            # bandwidth (one InstDMACopy spans all 16 SDMA engines) — no
            # fair-share mixing. Dependent stores (out) go on gpsimd so they
            # never head-of-line-block the load stream.
            for c in range(NCH):
                ksl = slice(c * KCH, (c + 1) * KCH)
                mn_c = []
                h = KCH // 2
                for et in range(NE):
                    mn = mapnat.tile(
                        [P, KCH, P], MM_DT, name=f"mn{et}_{c}", tag="mn"
                    )
                    if c == 0 and et == 0:
                        # split the very first block so PE can start sooner
                        nc.sync.dma_start(mn[:, :2, :], map_rs[0][:, :2, :])
                        nc.sync.dma_start(mn[:, 2:KCH, :], map_rs[0][:, 2:KCH, :])
                    else:
                        nc.sync.dma_start(mn[:], map_rs[et][:, ksl, :])
                    mn_c.append(mn)
                    # interleave doc halves so the first matmuls of the group
                    # don't wait behind a full 1MB doc transfer in the FIFO
                    if et == 0:
                        dsl = slice(c * KCH, c * KCH + h)
                        nc.sync.dma_start(doc_sb[:, dsl, :], doc_r[:, dsl, :])
                    elif et == 1:
                        dsl = slice(c * KCH + h, (c + 1) * KCH)
                        nc.sync.dma_start(doc_sb[:, dsl, :], doc_r[:, dsl, :])

                for et in range(NE):
                    mn = mn_c[et]
                    mt = mapt.tile([P, KCH, P], MM_DT, name=f"mt{et}_{c}", tag="mt")
                    for j in range(KCH):
                        ps_t = pst.tile([P, P], MM_DT)
                        nc.tensor.transpose(ps_t[:], mn[:, j, :], ident[:])
                        nc.vector.tensor_copy(mt[:, j, :], ps_t[:])
                    for j in range(KCH):
                        nc.tensor.matmul(
                            ps_os[et][:],
                            mt[:, j, :],
                            doc_sb[:, c * KCH + j, :],
                            start=(c == 0 and j == 0),
                            stop=(c == NCH - 1 and j == KCH - 1),
                        )

            # scale by 1/lens (per-partition scalar) and store
            for et in range(NE):
                ob = outp.tile([P, D], F32, name=f"ob{et}", tag="ob")
                nc.vector.tensor_scalar_mul(ob[:], ps_os[et][:], recip[:, et : et + 1])
                eng = nc.scalar if et % 2 == 0 else nc.sync
                eng.dma_start(out_d[et * P : (et + 1) * P, :], ob[:])

    nc.compile()
    return nc


_NC_CACHE = {}


def _get_nc():
    if "nc" not in _NC_CACHE:
        _NC_CACHE["nc"] = _build_bass()
    return _NC_CACHE["nc"]


def kernel(doc_state, entity_mapping, entity_lens, _trace=False, _tmpdir=None):
    doc_state = np.asarray(doc_state, dtype=np.float32)
    entity_mapping = np.asarray(entity_mapping, dtype=np.float32)
    entity_lens = np.asarray(entity_lens, dtype=np.float32)
    assert doc_state.shape == (B, L, D)
    assert entity_mapping.shape == (B, E, L)
    assert entity_lens.shape == (B, E)

    nc = _get_nc()

    in_maps = []
    for b in range(B):
        in_maps.append(
            {
                "map": np.ascontiguousarray(entity_mapping[b]),
                "doc": np.ascontiguousarray(doc_state[b]),
                # packed so lens[e] sits at partition e%128, column e//128
                "lens": np.ascontiguousarray(entity_lens[b].reshape(NE, P).T),
            }
        )

    res = bass_utils.run_bass_kernel_spmd(
        nc,
        in_maps,
        core_ids=list(range(N_CORES)),
        trace=_trace,
        tmpdir=_tmpdir,
    )
    out = np.stack([res.results[b]["out"] for b in range(B)], axis=0)
    if _trace:
        kernel.last_result = res
    return out
